# revision 1
# baseline (speedup 1.0000x reference)
"""CBAM attention kernel for Trainium2, 8-core data-parallel SPMD.

Layout per core: 4 samples of x [512c, 4096hw] fp32, c = 128*q + p (q in 0..3).
Per sample:
  - channel avg  : ScalarE activation(Copy, scale=1/4096, accum_out)   [stats col 2q]
  - channel max  : VectorE tensor_scalar(mult 1.0, op1=max, accum_out) [stats col 2q+1]
  - spatial sum  : PE ones-matmul accumulated over q -> c_row [1, 4096]
  - spatial max  : VectorE q-fold max -> PE 128-chunk transposes -> VectorE
                   segmented reduce_max -> mx_map [128, 32]  (p = 64r+w, col = h//2)
  - 7x7 conv     : 28 banded PE matmuls (host-precomputed band matrices,
                   /512 mean folded into the mean-path bands)
  - MLP          : tiny PE matmuls, relu/sigmoid on ScalarE
  - apply        : att = a (x) b outer-product on PE into PSUM,
                   out = x * att on VectorE (in-place into x tiles)
"""
import sys

sys.path.insert(0, "/opt/trn_rl_repo")
import numpy as np
import concourse.bass as bass
import concourse.bacc as bacc
import concourse.mybir as mybir
from concourse import tile
from concourse.bass_utils import run_bass_kernel_spmd

ALPHA = 0.02
NCORES = 8
B, C, H, W = 32, 512, 64, 64
HW = H * W          # 4096
SPC = B // NCORES   # 4 samples per core
F32 = mybir.dt.float32
AF = mybir.ActivationFunctionType
ALU = mybir.AluOpType
AX = mybir.AxisListType

# conv matmul emission order: dh=0 first so the PSUM group-start write covers
# the full column range (partial-range taps accumulate afterwards)
_DH_ORDER = [0, -1, 1, -2, 2, -3, 3]


def _emit_sample(nc, pools, dram, s):
    xd, outd = dram["x"], dram["out"]
    w1t_t, w2t_t, bands_t, ident_t, ones_t = (
        pools["w1t"], pools["w2t"], pools["bands"], pools["ident"], pools["ones"])
    xpool, cmpool, jpool, spool, mpool = (
        pools["xq"], pools["colmax"], pools["junk"], pools["small"], pools["maps"])
    rpool = pools["rows"]
    aux, tppool = pools["aux"], pools["tp"]

    # ---- load sample ----
    xq = []
    for q in range(4):
        t = xpool.tile([128, HW], F32, tag="xq")
        nc.sync.dma_start(t[:], xd[s, q])
        xq.append(t)

    stats = spool.tile([128, 8], F32, tag="stats")   # cols: avg0,max0,avg1,max1,...
    colmax = cmpool.tile([128, HW], F32, tag="colmax")
    junk_a = jpool["junk_a"]

    # ---- channel stats (A on ScalarE, B on VectorE; B scratch-writes colmax) ----
    for q in range(4):
        nc.scalar.activation(junk_a[:].rearrange("p (o b) -> p o b", o=1)
                             .broadcast_to([128, 8, 512]),
                             xq[q][:].rearrange("p (a b) -> p a b", b=512),
                             AF.Copy, bias=0.0,
                             scale=1.0 / HW, accum_out=stats[:, 2 * q:2 * q + 1])
        nc.vector.tensor_scalar(colmax[:], xq[q][:], 1.0, None,
                                op0=ALU.mult, op1=ALU.max,
                                accum_out=stats[:, 2 * q + 1:2 * q + 2])

    # ---- spatial sum over channels: PE ones-matmuls -> c_row [1, 4096] ----
    c_row = rpool.tile([1, HW], F32, tag="crow")
    for bk in range(8):
        cps = aux.tile([1, 512], F32, tag="aux")
        for q in range(4):
            nc.tensor.matmul(cps[:], ones_t[:, 0:1], xq[q][:, 512 * bk:512 * bk + 512],
                             start=(q == 0), stop=(q == 3))
        nc.scalar.copy(c_row[0:1, 512 * bk:512 * bk + 512], cps[:])

    # ---- spatial max: q-fold then transpose+reduce ----
    nc.vector.tensor_max(colmax[:], xq[0][:], xq[1][:])
    nc.vector.tensor_max(colmax[:], colmax[:], xq[2][:])
    nc.vector.tensor_max(colmax[:], colmax[:], xq[3][:])
    mx_map = mpool.tile([128, 32], F32, tag="mxmap")
    for g in range(8):
        tp = tppool.tile([128, 512], F32, tag="tp")
        for i in range(4):
            jj = 4 * g + i
            nc.tensor.transpose(tp[:, 128 * i:128 * i + 128],
                                colmax[:, 128 * jj:128 * jj + 128], ident_t[:])
        nc.vector.reduce_max(mx_map[:, 4 * g:4 * g + 4],
                             tp[:].rearrange("p (c x) -> p c x", x=128), axis=AX.X)

    # ---- mean map [128, 32] via reshape DMA: mm_map[p, j] = c_row[128j + p] ----
    mm_map = mpool.tile([128, 32], F32, tag="mmmap")
    nc.sync.dma_start(mm_map[:],
                      c_row[0:1, :].rearrange("o (j p) -> (o p) j", p=128))

    # ---- 7x7 conv as banded matmuls; sigmoid into bs_map ----
    # All matmul operands must sit at partition base 0: mixing base-0 and
    # base-64 operands in one PSUM accumulation group faults the device
    # (concurrent PE array tiles race on the same PSUM addresses). Copy the
    # odd-row halves of the maps down to base-0 tiles first.
    mm_hi = mpool.tile([64, 32], F32, tag="mmhi")
    nc.sync.dma_start(mm_hi[:], mm_map[64:128, :])
    mx_hi = mpool.tile([64, 32], F32, tag="mxhi")
    nc.sync.dma_start(mx_hi[:], mx_map[64:128, :])
    bs_map = mpool.tile([128, 32], F32, tag="bsmap")
    for r in range(2):
        cvp = aux.tile([64, 32], F32, tag="aux")
        n_ops = 14
        idx = 0
        for dh in _DH_ORDER:
            sh = r + dh
            r_in = sh % 2
            m = (sh - r_in) // 2
            jlo = max(0, -m)
            jhi = 32 - max(0, m)
            maps = (mm_map, mx_map) if r_in == 0 else (mm_hi, mx_hi)
            for mi, mp in enumerate(maps):
                lhsT = bands_t[0:64, mi * 7 + dh + 3, :]
                nc.tensor.matmul(cvp[:, jlo:jhi],
                                 lhsT, mp[0:64, jlo + m:jhi + m],
                                 start=(idx == 0), stop=(idx == n_ops - 1))
                idx += 1
        nc.scalar.activation(bs_map[64 * r:64 * r + 64, :], cvp[:], AF.Sigmoid)
    b_map = mpool.tile([128, 32], F32, tag="bmap")
    nc.vector.tensor_scalar(b_map[:], bs_map[:], ALPHA, 1.0 - ALPHA,
                            op0=ALU.mult, op1=ALU.add)

    # ---- channel MLP ----
    hps = aux.tile([32, 2], F32, tag="aux")
    for q in range(4):
        nc.tensor.matmul(hps[:], w1t_t[:, q, :], stats[:, 2 * q:2 * q + 2],
                         start=(q == 0), stop=(q == 3))
    hrelu = spool.tile([32, 2], F32, tag="hrelu")
    nc.scalar.activation(hrelu[:], hps[:], AF.Relu)
    hsum = spool.tile([32, 1], F32, tag="hsum")
    nc.vector.tensor_add(hsum[:], hrelu[:, 0:1], hrelu[:, 1:2])
    mcps = aux.tile([128, 4], F32, tag="aux")
    for q in range(4):
        nc.tensor.matmul(mcps[:, q:q + 1], w2t_t[:, 128 * q:128 * q + 128], hsum[:])
    sg = spool.tile([128, 4], F32, tag="sg")
    nc.scalar.activation(sg[:], mcps[:], AF.Sigmoid)
    a_col = spool.tile([128, 4], F32, tag="acol")
    nc.vector.tensor_scalar(a_col[:], sg[:], ALPHA, 1.0 - ALPHA,
                            op0=ALU.mult, op1=ALU.add)

    # ---- b back to hw-ordered row, then broadcast to all 128 partitions ----
    # (PE fp32 is 2-pass reduced precision -> the attention product must stay
    # on VectorE/DMA, which are exact fp32)
    # b_map[p, j] is b at hw = 128j + p. Bounce through DRAM, then one strided
    # load does transpose + broadcast: bb[p', 128j + p] = b_map[p, j] for all p'.
    dpool = pools["dram"]
    bscr = dpool.tile([1, 32, 128], F32, tag="bscr")   # hw-ordered b row in DRAM
    nc.sync.dma_start(bscr[0, :, :].rearrange("j p -> p j"), b_map[:, :])
    bb = rpool.tile([128, HW], F32, tag="bb")
    nc.sync.dma_start(bb[:],
                      bscr[:, :, :].rearrange("o j p -> o (j p)")
                      .broadcast_to([128, HW]))

    # ---- apply attention on VectorE: x *= a[c] (tensor_scalar, 2x mode),
    #      then x *= b[hw] (tensor_tensor vs broadcast row) ----
    for q in range(4):
        nc.vector.tensor_scalar(xq[q][:], xq[q][:], a_col[:, q:q + 1], None,
                                op0=ALU.mult)
        nc.vector.tensor_mul(xq[q][:], xq[q][:], bb[:])
        nc.sync.dma_start(outd[s, q], xq[q][:])


def build_nc(spc=SPC):
    nc = bacc.Bacc("TRN2", target_bir_lowering=False, debug=False)
    dram = {
        "x": nc.declare_dram_parameter("x", [spc, 4, 128, HW], F32, isOutput=False),
        "w1t": nc.declare_dram_parameter("w1t", [128, 4, 32], F32, isOutput=False),
        "w2t": nc.declare_dram_parameter("w2t", [32, 512], F32, isOutput=False),
        "bands": nc.declare_dram_parameter("bands", [128, 14, 64], F32, isOutput=False),
        "ident": nc.declare_dram_parameter("ident", [128, 128], F32, isOutput=False),
        "ones": nc.declare_dram_parameter("ones", [128, 1], F32, isOutput=False),
        "out": nc.declare_dram_parameter("out", [spc, 4, 128, HW], F32, isOutput=True),
    }
    with tile.TileContext(nc) as tc:
        with (
            tc.tile_pool(name="const", bufs=1) as cpool,
            tc.tile_pool(name="xq", bufs=8) as xpool,
            tc.tile_pool(name="colmax", bufs=1) as cmpool,
            tc.tile_pool(name="junk", bufs=1) as jpool,
            tc.tile_pool(name="small", bufs=2) as spool,
            tc.tile_pool(name="maps", bufs=2) as mpool,
            tc.tile_pool(name="rows", bufs=1) as rpool,
            tc.tile_pool(name="dram", bufs=2, space="DRAM") as dpool,
            tc.tile_pool(name="aux", bufs=2, space="PSUM") as aux,
            tc.tile_pool(name="tp", bufs=2, space="PSUM") as tppool,
        ):
            pools = {
                "xq": xpool, "colmax": cmpool, "small": spool, "maps": mpool,
                "aux": aux, "tp": tppool, "rows": rpool, "dram": dpool,
                "w1t": cpool.tile([128, 4, 32], F32, tag="w1t", name="w1t_sb"),
                "w2t": cpool.tile([32, 512], F32, tag="w2t", name="w2t_sb"),
                "bands": cpool.tile([128, 14, 64], F32, tag="bands", name="bands_sb"),
                "ident": cpool.tile([128, 128], F32, tag="ident", name="ident_sb"),
                "ones": cpool.tile([128, 1], F32, tag="ones", name="ones_sb"),
                "junk": {"junk_a": jpool.tile([128, 512], F32, tag="junk_a",
                                              name="junk_a")},
            }
            for name in ("w1t", "w2t", "bands", "ident", "ones"):
                nc.sync.dma_start(pools[name][:], dram[name][:])
            for s in range(spc):
                _emit_sample(nc, pools, dram, s)
    nc.compile()
    return nc


def make_consts(w1, w2, wconv):
    w1t = np.ascontiguousarray(
        w1.T.reshape(4, 128, 32).transpose(1, 0, 2)).astype(np.float32)
    w2t = np.ascontiguousarray(w2.T).astype(np.float32)
    bands = np.zeros((2, 7, 64, 64), np.float32)
    for ci in range(2):
        k = wconv[0, ci]
        for dh in range(7):
            for dw in range(7):
                diag = dw - 3  # w_in - w_out
                v = np.float32(k[dh, dw])
                idx = np.arange(max(0, -diag), min(64, 64 - diag))  # w_out range
                bands[ci, dh, idx + diag, idx] = v
    bands[0] /= 512.0
    bands_r = np.ascontiguousarray(
        bands.transpose(2, 0, 1, 3).reshape(64, 14, 64)).astype(np.float32)
    bands_r = np.ascontiguousarray(np.concatenate([bands_r, bands_r], axis=0))
    ident = np.eye(128, dtype=np.float32)
    ones = np.ones((128, 1), np.float32)
    return {"w1t": w1t, "w2t": w2t, "bands": bands_r, "ident": ident, "ones": ones}


_NC = None


def kernel(**inputs):
    global _NC
    x = np.ascontiguousarray(np.asarray(inputs["x"], dtype=np.float32))
    w1 = np.asarray(inputs["w1"], dtype=np.float32)
    w2 = np.asarray(inputs["w2"], dtype=np.float32)
    wconv = np.asarray(inputs["wconv"], dtype=np.float32)

    if _NC is None:
        _NC = build_nc()
    consts = make_consts(w1, w2, wconv)
    shards = x.reshape(NCORES, SPC, 4, 128, HW)
    in_maps = [dict(consts, x=np.ascontiguousarray(shards[i]))
               for i in range(NCORES)]
    res = run_bass_kernel_spmd(_NC, in_maps, core_ids=list(range(NCORES)))
    out = np.concatenate(
        [np.asarray(res.results[i]["out"]).reshape(SPC, C, H, W)
         for i in range(NCORES)], axis=0)
    return out.astype(np.float32)

